# revision 11
# baseline (speedup 1.0000x reference)
"""DopDense forward: relu(x @ (w * mult) + b) on 8 trn2 NeuronCores.

Key algebra: w_new = w * mult (per-column scaling) commutes with the matmul,
so out = relu((x @ w) * mult[None, :] + b).  We compute y^T tiles (units on
partitions, batch on free axis) so the per-column mult/bias become
per-partition scale/bias of a fused Relu eviction (scalar-engine activation
or a 2-op vector tensor_scalar).

mult is computed on device: dd[j] = sum_i |w[i,d_j] - old[i,d_j]| (vector
engine), gating logic in j-space, then a multiplicative scatter to columns
as mult = (1 + L^T lfm1) * (1 + R^T rfm1) -- left/right target columns are
each unique, and the single collision (column 0) is handled exactly by the
product.  L/R are built on device from an iota constant via is_equal.

Sharding: data-parallel over the batch axis (8192 rows/core); w, dop state
replicated.  The big matmul runs in bf16 and the output is stored in bf16
(upconverted on host), so the kernel is tensor-engine bound (~55us of
matmul issue) with DMA (~18 MB/core) fully overlapped underneath.

Schedule: the DMA queues ramp slowly (~0.3 GB/ns aggregate over the first
10us), so the first two xt windows are processed as k-OUTER half-window
solos -- each [128,512] psum group consumes one 128KB x chunk per 0.86us,
matching what the queues can actually deliver, and the matmul stream starts
~11us in.  The middle four windows run as c-outer window pairs (max
stationary reuse).  The last two windows are again k-outer solos so the
final psum groups stop early and the tail is one small eviction + DMA.
The dop scatter matmuls write a tiny separate PSUM tag, decoupling the
mult critical path from the main psum rotation.
"""

import numpy as np
import ml_dtypes


def _install_ntff_shim():
    """The trimmed antenv package in this image lacks axon_hooks, which
    concourse's trace=True path imports unconditionally.  Recreate the hook
    registry (and install the ctypes NTFF hook when available) so tracing
    works whether or not the caller enables it."""
    import sys
    import types
    try:
        import antenv
        import antenv.axon_hooks  # noqa: F401
        return
    except ImportError:
        pass
    try:
        import antenv
    except ImportError:
        return
    mod = types.ModuleType("antenv.axon_hooks")
    holder = [None]
    try:
        from trn_agent_boot.trn_boot import _ntff_profile_via_ctypes
        holder[0] = _ntff_profile_via_ctypes("/opt/axon/libaxon_pjrt.so")
    except Exception:
        pass
    mod.get_axon_ntff_profile_hook = lambda: holder[0]
    mod.set_axon_ntff_profile_hook = lambda h: holder.__setitem__(0, h)
    sys.modules["antenv.axon_hooks"] = mod
    antenv.axon_hooks = mod


_install_ntff_shim()

import concourse.bass as bass
import concourse.mybir as mybir
import concourse.tile as tile
from concourse import bacc
from concourse.bass_utils import run_bass_kernel_spmd

F32 = mybir.dt.float32
BF16 = mybir.dt.bfloat16
AF = mybir.ActivationFunctionType
ALU = mybir.AluOpType
BF16_NP = np.dtype(ml_dtypes.bfloat16)

N_CORES = 8
B = 65536
NIN = 512
UNITS = 512
N_DOP = 128
SHARD = B // N_CORES          # 8192 batch rows per core
W = 1024                      # xt window size (batch cols per dram slice)
NWP = SHARD // W              # 8 xt windows per core
KC = NIN // 128               # 4 contraction chunks
CC = UNITS // 128             # 4 unit chunks
THRESHOLD = 0.0
REF_PERIOD = 2.0
NWARM = 7                     # PE clock warm-up matmuls

# w chunks packed k-major (k-outer solos consume (k, c=0..3) together)
ORDER = [(k, c) for k in range(KC) for c in range(CC)]
WKPOS = {kc: i for i, kc in enumerate(ORDER)}

# Static dopaminergic-column index math (mirrors reference.py exactly)
DOP_IDX = np.linspace(1, UNITS - 1, N_DOP, dtype=np.int32)
LEFT_OK = ~np.isin(DOP_IDX - 1, DOP_IDX)
RIGHT_OK = ~np.isin(DOP_IDX + 1, DOP_IDX)
LCOL = (DOP_IDX - 1) % UNITS
RCOL = (DOP_IDX + 1) % UNITS

LOK10 = LEFT_OK.astype(np.float32) * np.float32(10.0 / NIN)
ROK10 = RIGHT_OK.astype(np.float32) * np.float32(10.0 / NIN)

_CACHED_NC = None


def build_nc():
    global _CACHED_NC
    if _CACHED_NC is not None:
        return _CACHED_NC
    nc = bacc.Bacc("TRN2", target_bir_lowering=False, debug=False,
                   num_swdge_queues=2)

    xt = nc.dram_tensor("xt", [NWP, 128, KC * W], BF16, kind="ExternalInput")
    wkb = nc.dram_tensor("wkb", [128, KC * CC * 128], BF16, kind="ExternalInput")
    # aux inputs packed into one wide tensor (small-row DMAs are slow):
    # [:, 0:18] = per-partition vectors (lok10, rok10, indicator, batch_ctr,
    # b0..b3, lcol%128, rcol%128, Lchunkmask[4], Rchunkmask[4]),
    # [:, 18:146] = iota rows
    NV = 18
    auxs = nc.dram_tensor("auxs", [128, NV + 128], F32, kind="ExternalInput")
    # dop columns of w^T and old^T in bf16 (the |w-old| sum over 512 terms
    # is insensitive to bf16 rounding; halves the critical aux DMA)
    auxb = nc.dram_tensor("auxb", [128, 2 * NIN], BF16, kind="ExternalInput")
    # output in bf16 (rel-err budget 2e-2; bf16 adds ~2e-3) -- halves the
    # dominant output DMA traffic. Host upconverts to fp32.
    yt = nc.dram_tensor("yt", [NWP, 128, CC * W], BF16, kind="ExternalOutput")

    with tile.TileContext(nc) as tc:
        with (
            tc.tile_pool(name="const", bufs=1) as const,
            tc.tile_pool(name="aux", bufs=1) as aux,
            tc.tile_pool(name="xa", bufs=4) as xpool,
            tc.tile_pool(name="ob", bufs=4) as opool,
            tc.tile_pool(name="tmp", bufs=2) as tpool,
        ):
            wk_sb = const.tile([128, KC * CC * 128], BF16, tag="wk")
            axs_sb = const.tile([128, NV + 128], F32, tag="axs")
            axb_sb = const.tile([128, 2 * NIN], BF16, tag="axb")
            xa_tiles = {}

            def wk_row(k):
                # one k-row of stationary chunks: (k, c=0..3) = 512 cols
                return (wk_sb[:, k * 512:(k + 1) * 512],
                        wkb[:, k * 512:(k + 1) * 512])

            def wk_tile(k, c):
                i = WKPOS[(k, c)]
                return wk_sb[:, i * 128:(i + 1) * 128]

            # ---------- input DMAs: emission order = per-queue priority ----
            # The head solos consume x at k-chunk granularity; their chunks
            # (and the k-rows of w) are striped across all three queues in
            # first-use order.  aux leads the scalar queue (it gates mult ->
            # every eviction).
            xa0 = xpool.tile([128, KC * W], BF16, tag="xa")
            xa1 = xpool.tile([128, KC * W], BF16, tag="xa")
            xa_tiles[0], xa_tiles[1] = xa0, xa1

            def xslice(eng, xa, wp, k, half):
                o = k * W + half * 512
                eng.dma_start(xa[:, o:o + 512], xt[wp][:, o:o + 512])

            nc.sync.dma_start(*wk_row(0))
            xslice(nc.sync, xa0, 0, 0, 0)        # solo0 k0
            nc.gpsimd.dma_start(*wk_row(1))
            xslice(nc.gpsimd, xa0, 0, 1, 0)      # solo0 k1
            nc.scalar.dma_start(axs_sb[:], auxs[:])
            nc.scalar.dma_start(axb_sb[:], auxb[:])
            xslice(nc.sync, xa0, 0, 2, 0)        # solo0 k2
            nc.gpsimd.dma_start(*wk_row(2))
            xslice(nc.gpsimd, xa0, 0, 3, 0)      # solo0 k3
            nc.scalar.dma_start(*wk_row(3))
            # solo1 = right half of xt0
            xslice(nc.sync, xa0, 0, 0, 1)
            xslice(nc.scalar, xa0, 0, 1, 1)
            xslice(nc.gpsimd, xa0, 0, 2, 1)
            xslice(nc.sync, xa0, 0, 3, 1)
            # solos 2/3 = xt1 halves
            xslice(nc.scalar, xa1, 1, 0, 0)
            xslice(nc.gpsimd, xa1, 1, 1, 0)
            xslice(nc.sync, xa1, 1, 2, 0)
            xslice(nc.scalar, xa1, 1, 3, 0)
            xslice(nc.gpsimd, xa1, 1, 0, 1)
            xslice(nc.sync, xa1, 1, 1, 1)
            xslice(nc.gpsimd, xa1, 1, 2, 1)
            xslice(nc.scalar, xa1, 1, 3, 1)

            def load_xa(wp, eng):
                xa = xpool.tile([128, KC * W], BF16, tag="xa")
                eng.dma_start(xa[:, :2 * W], xt[wp][:, :2 * W])
                eng.dma_start(xa[:, 2 * W:], xt[wp][:, 2 * W:])
                xa_tiles[wp] = xa

            v_sb = axs_sb[:, 0:NV]
            io_sb = axs_sb[:, NV:NV + 128]
            wd_sb = axb_sb[:, 0:NIN]
            od_sb = axb_sb[:, NIN:2 * NIN]

            # PE warm-up scratch (vector memset; queues are busy with x)
            scr = const.tile([128, 512], BF16, tag="scr")
            nc.vector.memset(scr[:], 0.0)

            # scatter masks from iota: Lmod[j, m] = 1 iff LCOL[j] % 128 == m
            lmod = const.tile([128, 128], BF16, tag="lmod")
            nc.vector.tensor_scalar(lmod[:], io_sb, v_sb[:, 8:9],
                                    None, op0=ALU.is_equal)
            rmod = const.tile([128, 128], BF16, tag="rmod")
            nc.vector.tensor_scalar(rmod[:], io_sb, v_sb[:, 9:10],
                                    None, op0=ALU.is_equal)

            # ---------- aux compute: dd[j] = sum_i |w[i,d_j] - old[i,d_j]| --
            dch = aux.tile([128, NIN], F32, tag="dch")
            nc.vector.tensor_tensor(dch[:], wd_sb, od_sb, op=ALU.subtract)
            dd = const.tile([128, 1], F32, tag="dd")
            nc.vector.tensor_reduce(
                dd[:], dch[:], axis=mybir.AxisListType.X, op=ALU.add,
                apply_absolute_value=True,
            )
            # active = (dd > THRESHOLD) & ((batch_ctr - indicator) > REF_PERIOD)
            t1 = const.tile([128, 1], F32, tag="t1")
            nc.vector.tensor_tensor(t1[:], v_sb[:, 3:4], v_sb[:, 2:3],
                                    op=ALU.subtract)
            c2 = const.tile([128, 1], F32, tag="c2")
            nc.vector.tensor_scalar(c2[:], t1[:], REF_PERIOD, None, op0=ALU.is_gt)
            c1 = const.tile([128, 1], F32, tag="c1")
            nc.vector.tensor_scalar(c1[:], dd[:], THRESHOLD, None, op0=ALU.is_gt)
            av = const.tile([128, 1], F32, tag="av")
            nc.vector.tensor_tensor(av[:], c1[:], c2[:], op=ALU.mult)
            da = const.tile([128, 1], F32, tag="da")
            nc.vector.tensor_tensor(da[:], dd[:], av[:], op=ALU.mult)
            lf1 = const.tile([128, 1], F32, tag="lf1")
            nc.vector.tensor_tensor(lf1[:], da[:], v_sb[:, 0:1], op=ALU.mult)
            rf1 = const.tile([128, 1], F32, tag="rf1")
            nc.vector.tensor_tensor(rf1[:], da[:], v_sb[:, 1:2], op=ALU.mult)

            lfc = const.tile([128, CC], BF16, tag="lfc")
            nc.vector.tensor_scalar(lfc[:], v_sb[:, 10:10 + CC], lf1[:],
                                    None, op0=ALU.mult)
            rfc = const.tile([128, CC], BF16, tag="rfc")
            nc.vector.tensor_scalar(rfc[:], v_sb[:, 14:14 + CC], rf1[:],
                                    None, op0=ALU.mult)
            multm = const.tile([128, CC], F32, tag="multm")
            mult_sb = [multm[:, cc:cc + 1] for cc in range(CC)]

            # ---------- main: y^T = (w^T x^T) scaled+biased+relu ----------
            def evict_act(ps, ob, c, half):
                o = c * W + half * 512
                nc.scalar.activation(
                    ob[:, o:o + 512], ps[:], AF.Relu,
                    bias=v_sb[:, 4 + c:5 + c], scale=mult_sb[c])

            def evict_dve(ps, ob, c, half):
                o = c * W + half * 512
                tmp = tpool.tile([128, 512], F32, tag="evt")
                nc.vector.tensor_scalar(
                    tmp[:], ps[:], mult_sb[c], v_sb[:, 4 + c:5 + c],
                    op0=ALU.mult, op1=ALU.add)
                nc.vector.tensor_scalar(
                    ob[:, o:o + 512], tmp[:], 0.0, None, op0=ALU.max)

            with tc.tile_pool(name="ps", bufs=1, space="PSUM") as pspool:
                # PE clock warm-up burst bridges until the first x chunk
                warm = pspool.tile([128, 512], F32, tag="m5", bufs=6)
                for _ in range(NWARM):
                    nc.tensor.matmul(warm[:], scr[:, :128], scr[:],
                                     start=True, stop=True)

                def aux_scatter():
                    # dop scatter matmuls; tiny separate psum tag so the
                    # mult critical path never touches the main rotation
                    psl = pspool.tile([128, CC], F32, tag="auxps", bufs=2)
                    nc.tensor.matmul(psl[:], lmod[:], lfc[:],
                                     start=True, stop=True)
                    psr = pspool.tile([128, CC], F32, tag="auxps", bufs=2)
                    nc.tensor.matmul(psr[:], rmod[:], rfc[:],
                                     start=True, stop=True)
                    lsp = const.tile([128, CC], F32, tag="lsp")
                    nc.vector.tensor_scalar(lsp[:], psl[:], 1.0, None,
                                            op0=ALU.add)
                    rsp = const.tile([128, CC], F32, tag="rsp")
                    nc.vector.tensor_scalar(rsp[:], psr[:], 1.0, None,
                                            op0=ALU.add)
                    nc.vector.tensor_tensor(multm[:], lsp[:], rsp[:],
                                            op=ALU.mult)

                def solo(wp, half, first=False, ytail=None):
                    # k-outer half-window: 4 open [128,512] psum groups;
                    # consumes one 128KB x chunk per 4 matmuls
                    xa = xa_tiles[wp]
                    if wp not in solo_ob:
                        solo_ob[wp] = opool.tile([128, CC * W], BF16,
                                                 tag="ob", name=f"sob{wp}")
                    ob = solo_ob[wp]
                    pcs = [pspool.tile([128, 512], F32, tag="m5", bufs=6,
                                       name=f"pc{wp}_{half}_{c}")
                           for c in range(CC)]
                    for ki in range(KC):
                        for c in range(CC):
                            nc.tensor.matmul(
                                pcs[c][:], wk_tile(ki, c),
                                xa[:, ki * W + half * 512:
                                   ki * W + half * 512 + 512],
                                start=(ki == 0), stop=(ki == KC - 1))
                        if first and ki == KC - 2:
                            aux_scatter()
                    for c in range(CC):
                        if c % 2 == 0:
                            evict_act(pcs[c], ob, c, half)
                        else:
                            evict_dve(pcs[c], ob, c, half)
                        eng = ytail[c] if ytail is not None else (
                            nc.scalar if c < 2 else
                            (nc.gpsimd if c == 2 else nc.sync))
                        o = c * W + half * 512
                        eng.dma_start(yt[wp][:, o:o + 512], ob[:, o:o + 512])

                def pair(wpa, wpb, loads=()):
                    for wp, eng in loads:
                        load_xa(wp, eng)
                    xaa, xab = xa_tiles[wpa], xa_tiles[wpb]
                    oba = opool.tile([128, CC * W], BF16, tag="ob")
                    obb = opool.tile([128, CC * W], BF16, tag="ob")
                    for c in range(CC):
                        ps4 = [pspool.tile([128, 512], F32, tag="m5", bufs=6,
                                           name=f"ps{wpa}_{c}_{i}")
                               for i in range(4)]
                        korder = range(KC) if c % 2 == 0 \
                            else range(KC - 1, -1, -1)
                        for ki, k in enumerate(korder):
                            for i, xa in ((0, xaa), (2, xab)):
                                for s in range(2):
                                    nc.tensor.matmul(
                                        ps4[i + s][:], wk_tile(k, c),
                                        xa[:, k * W + s * 512:
                                           k * W + (s + 1) * 512],
                                        start=(ki == 0), stop=(ki == KC - 1))
                        # s0 -> scalar ACT, s1 -> vector DVE (parallel)
                        evict_act(ps4[0], oba, c, 0)
                        evict_dve(ps4[1], oba, c, 1)
                        evict_act(ps4[2], obb, c, 0)
                        evict_dve(ps4[3], obb, c, 1)
                        enga = nc.scalar if c < 2 else (
                            nc.gpsimd if c == 2 else nc.sync)
                        enga.dma_start(yt[wpa][:, c * W:(c + 1) * W],
                                       oba[:, c * W:(c + 1) * W])
                        enga.dma_start(yt[wpb][:, c * W:(c + 1) * W],
                                       obb[:, c * W:(c + 1) * W])

                solo_ob = {}
                # head: xt0/xt1 as k-outer half-window solos; prefetch
                # the pair windows behind them
                solo(0, 0, first=True)
                load_xa(2, nc.sync)
                load_xa(3, nc.gpsimd)
                solo(0, 1)
                load_xa(4, nc.sync)
                load_xa(5, nc.gpsimd)
                solo(1, 0)
                load_xa(6, nc.sync)
                load_xa(7, nc.gpsimd)
                solo(1, 1)
                # middle: c-outer pairs (max stationary reuse)
                pair(2, 3)
                pair(4, 5)
                # tail: k-outer solos; keep the slow-draining SWDGE queue
                # away from the kernel end, alternate sync/scalar drains
                TS = {0: nc.scalar, 1: nc.sync, 2: nc.scalar, 3: nc.sync}
                TS2 = {0: nc.sync, 1: nc.scalar, 2: nc.sync, 3: nc.scalar}
                solo(6, 0, ytail=TS)
                solo(6, 1, ytail=TS2)
                solo(7, 0, ytail=TS)
                solo(7, 1, ytail=TS2)

    nc.compile()
    _CACHED_NC = nc
    return nc


LAST_RESULTS = None


def kernel(x, w, b, dop_weights_old, indicator, batch_ctr):
    global LAST_RESULTS
    x = np.asarray(x, dtype=np.float32)
    w = np.ascontiguousarray(np.asarray(w, dtype=np.float32))
    b_arr = np.asarray(b, dtype=np.float32)
    old = np.asarray(dop_weights_old, dtype=np.float32)
    ind = np.asarray(indicator, dtype=np.float32)
    bc_val = float(np.asarray(batch_ctr).item())

    nc = build_nc()

    # replicated (per-core identical) inputs; all reshapes/gathers are pure
    # data marshaling -- every arithmetic op happens on device
    w4 = w.reshape(KC, 128, CC, 128)
    wkb = np.ascontiguousarray(np.concatenate(
        [w4[k, :, c, :] for (k, c) in ORDER], axis=1)).astype(BF16_NP)
    vcols = [LOK10, ROK10, ind.astype(np.float32),
             np.full(128, bc_val, np.float32)]
    vcols += [b_arr[c * 128:(c + 1) * 128] for c in range(CC)]
    vcols += [(LCOL % 128).astype(np.float32), (RCOL % 128).astype(np.float32)]
    vcols += [(LCOL // 128 == cc).astype(np.float32) for cc in range(CC)]
    vcols += [(RCOL // 128 == cc).astype(np.float32) for cc in range(CC)]
    vecs = np.stack(vcols, axis=1).astype(np.float32)
    iot = np.broadcast_to(np.arange(128, dtype=np.float32), (128, 128))
    auxs = np.ascontiguousarray(np.concatenate(
        [vecs, iot], axis=1, dtype=np.float32))
    auxb = np.ascontiguousarray(np.concatenate(
        [w.T[DOP_IDX], old.T[DOP_IDX]], axis=1, dtype=np.float32)
    ).astype(BF16_NP)

    common = dict(wkb=wkb, auxs=auxs, auxb=auxb)

    xbf = x.astype(BF16_NP)
    in_maps = []
    for i in range(N_CORES):
        xs = xbf[i * SHARD:(i + 1) * SHARD]          # [8192, 512]
        xtc = np.ascontiguousarray(
            xs.reshape(NWP, W, KC, 128).transpose(0, 3, 2, 1)
        ).reshape(NWP, 128, KC * W)
        in_maps.append(dict(common, xt=xtc))

    res = run_bass_kernel_spmd(nc, in_maps, core_ids=list(range(N_CORES)))
    LAST_RESULTS = res

    out = np.empty((B, UNITS), np.float32)
    for i in range(N_CORES):
        ytc = res.results[i]["yt"].astype(np.float32).reshape(NWP, 128, CC, W)
        out[i * SHARD:(i + 1) * SHARD] = (
            ytc.transpose(0, 3, 2, 1).reshape(SHARD, UNITS))
    return out


# revision 15
# speedup vs baseline: 1.0190x; 1.0190x over previous
"""DopDense forward: relu(x @ (w * mult) + b) on 8 trn2 NeuronCores.

Key algebra: w_new = w * mult (per-column scaling) commutes with the matmul,
so out = relu((x @ w) * mult[None, :] + b).  We compute y^T tiles (units on
partitions, batch on free axis) so the per-column mult/bias become
per-partition scale/bias of a fused Relu eviction (scalar-engine activation
or a 2-op vector tensor_scalar).

mult is computed on device: dd[j] = sum_i |w[i,d_j] - old[i,d_j]| (vector
engine), gating logic in j-space, then a multiplicative scatter to columns
as mult = (1 + L^T lfm1) * (1 + R^T rfm1) -- left/right target columns are
each unique, and the single collision (column 0) is handled exactly by the
product.  L/R are built on device from an iota constant via is_equal.

Sharding: data-parallel over the batch axis (8192 rows/core); w, dop state
replicated.  The big matmul runs in bf16 and the output is stored in bf16
(upconverted on host), so the kernel is tensor-engine bound (~55us of
matmul issue) with DMA (~18 MB/core) fully overlapped underneath.

Schedule: the DMA queues ramp slowly (~0.3 GB/ns aggregate over the first
10us), so the first two xt windows are processed as k-OUTER half-window
solos -- each [128,512] psum group consumes one 128KB x chunk per 0.86us,
matching what the queues can actually deliver, and the matmul stream starts
~11us in.  The middle four windows run as c-outer window pairs (max
stationary reuse).  The last two windows are again k-outer solos so the
final psum groups stop early and the tail is one small eviction + DMA.
The dop scatter matmuls write a tiny separate PSUM tag, decoupling the
mult critical path from the main psum rotation.
"""

import numpy as np
import ml_dtypes


def _install_ntff_shim():
    """The trimmed antenv package in this image lacks axon_hooks, which
    concourse's trace=True path imports unconditionally.  Recreate the hook
    registry (and install the ctypes NTFF hook when available) so tracing
    works whether or not the caller enables it."""
    import sys
    import types
    try:
        import antenv
        import antenv.axon_hooks  # noqa: F401
        return
    except ImportError:
        pass
    try:
        import antenv
    except ImportError:
        return
    mod = types.ModuleType("antenv.axon_hooks")
    holder = [None]
    try:
        from trn_agent_boot.trn_boot import _ntff_profile_via_ctypes
        holder[0] = _ntff_profile_via_ctypes("/opt/axon/libaxon_pjrt.so")
    except Exception:
        pass
    mod.get_axon_ntff_profile_hook = lambda: holder[0]
    mod.set_axon_ntff_profile_hook = lambda h: holder.__setitem__(0, h)
    sys.modules["antenv.axon_hooks"] = mod
    antenv.axon_hooks = mod


_install_ntff_shim()

import concourse.bass as bass
import concourse.mybir as mybir
import concourse.tile as tile
from concourse import bacc
from concourse.bass_utils import run_bass_kernel_spmd

F32 = mybir.dt.float32
BF16 = mybir.dt.bfloat16
AF = mybir.ActivationFunctionType
ALU = mybir.AluOpType
BF16_NP = np.dtype(ml_dtypes.bfloat16)

N_CORES = 8
B = 65536
NIN = 512
UNITS = 512
N_DOP = 128
SHARD = B // N_CORES          # 8192 batch rows per core
W = 1024                      # xt window size (batch cols per dram slice)
NWP = SHARD // W              # 8 xt windows per core
KC = NIN // 128               # 4 contraction chunks
CC = UNITS // 128             # 4 unit chunks
THRESHOLD = 0.0
REF_PERIOD = 2.0
NWARM = 8                     # PE clock warm-up matmuls

# w chunks packed k-major (k-outer solos consume (k, c=0..3) together)
ORDER = [(k, c) for k in range(KC) for c in range(CC)]
WKPOS = {kc: i for i, kc in enumerate(ORDER)}

# Static dopaminergic-column index math (mirrors reference.py exactly)
DOP_IDX = np.linspace(1, UNITS - 1, N_DOP, dtype=np.int32)
LEFT_OK = ~np.isin(DOP_IDX - 1, DOP_IDX)
RIGHT_OK = ~np.isin(DOP_IDX + 1, DOP_IDX)
LCOL = (DOP_IDX - 1) % UNITS
RCOL = (DOP_IDX + 1) % UNITS

LOK10 = LEFT_OK.astype(np.float32) * np.float32(10.0 / NIN)
ROK10 = RIGHT_OK.astype(np.float32) * np.float32(10.0 / NIN)

_CACHED_NC = None


def build_nc():
    global _CACHED_NC
    if _CACHED_NC is not None:
        return _CACHED_NC
    nc = bacc.Bacc("TRN2", target_bir_lowering=False, debug=False,
                   num_swdge_queues=2)

    xt = nc.dram_tensor("xt", [NWP, 128, KC * W], BF16, kind="ExternalInput")
    wkb = nc.dram_tensor("wkb", [128, KC * CC * 128], BF16, kind="ExternalInput")
    # aux inputs packed into one wide tensor (small-row DMAs are slow):
    # [:, 0:18] = per-partition vectors (lok10, rok10, indicator, batch_ctr,
    # b0..b3, lcol%128, rcol%128, Lchunkmask[4], Rchunkmask[4]),
    # [:, 18:146] = iota rows
    NV = 18
    auxs = nc.dram_tensor("auxs", [128, NV + 128], F32, kind="ExternalInput")
    # dop columns of w^T and old^T in bf16 (the |w-old| sum over 512 terms
    # is insensitive to bf16 rounding; halves the critical aux DMA)
    auxb = nc.dram_tensor("auxb", [128, 2 * NIN], BF16, kind="ExternalInput")
    # output in bf16 (rel-err budget 2e-2; bf16 adds ~2e-3) -- halves the
    # dominant output DMA traffic. Host upconverts to fp32.
    yt = nc.dram_tensor("yt", [NWP, 128, CC * W], BF16, kind="ExternalOutput")

    with tile.TileContext(nc) as tc:
        with (
            tc.tile_pool(name="const", bufs=1) as const,
            tc.tile_pool(name="aux", bufs=1) as aux,
            tc.tile_pool(name="xa", bufs=4) as xpool,
            tc.tile_pool(name="ob", bufs=4) as opool,
            tc.tile_pool(name="tmp", bufs=2) as tpool,
        ):
            wk_sb = const.tile([128, KC * CC * 128], BF16, tag="wk")
            axs_sb = const.tile([128, NV + 128], F32, tag="axs")
            axb_sb = const.tile([128, 2 * NIN], BF16, tag="axb")
            xa_tiles = {}

            def wk_row(k):
                # one k-row of stationary chunks: (k, c=0..3) = 512 cols
                return (wk_sb[:, k * 512:(k + 1) * 512],
                        wkb[:, k * 512:(k + 1) * 512])

            def wk_tile(k, c):
                i = WKPOS[(k, c)]
                return wk_sb[:, i * 128:(i + 1) * 128]

            # ---------- input DMAs: emission order = per-queue priority ----
            # The head solos consume x at k-chunk granularity; their chunks
            # (and the k-rows of w) are striped across all three queues in
            # first-use order.  aux leads the scalar queue (it gates mult ->
            # every eviction).
            xa0 = xpool.tile([128, KC * W], BF16, tag="xa")
            xa_tiles[0] = xa0

            def xslice(eng, xa, wp, k, half):
                o = k * W + half * 512
                eng.dma_start(xa[:, o:o + 512], xt[wp][:, o:o + 512])

            # xt0 halves arrive just-in-time for the first two k-outer
            # solos, k-chunks striped over sync/gpsimd; wk rows k-major;
            # aux on scalar (it gates mult -> every eviction)
            nc.sync.dma_start(*wk_row(0))
            xslice(nc.sync, xa0, 0, 0, 0)        # solo0 k0
            nc.gpsimd.dma_start(*wk_row(1))
            xslice(nc.gpsimd, xa0, 0, 1, 0)      # solo0 k1
            nc.scalar.dma_start(*wk_row(2))
            xslice(nc.sync, xa0, 0, 2, 0)        # solo0 k2
            nc.scalar.dma_start(axs_sb[:], auxs[:])
            xslice(nc.gpsimd, xa0, 0, 3, 0)      # solo0 k3
            nc.scalar.dma_start(axb_sb[:], auxb[:])
            xslice(nc.sync, xa0, 0, 0, 1)        # solo1 k0..k3
            xslice(nc.gpsimd, xa0, 0, 1, 1)
            nc.scalar.dma_start(*wk_row(3))
            xslice(nc.sync, xa0, 0, 2, 1)
            xslice(nc.gpsimd, xa0, 0, 3, 1)

            def load_xa(wp, enga, engb):
                xa = xpool.tile([128, KC * W], BF16, tag="xa")
                enga.dma_start(xa[:, :2 * W], xt[wp][:, :2 * W])
                engb.dma_start(xa[:, 2 * W:], xt[wp][:, 2 * W:])
                xa_tiles[wp] = xa

            load_xa(1, nc.sync, nc.gpsimd)

            v_sb = axs_sb[:, 0:NV]
            io_sb = axs_sb[:, NV:NV + 128]
            wd_sb = axb_sb[:, 0:NIN]
            od_sb = axb_sb[:, NIN:2 * NIN]

            # PE warm-up scratch (vector memset; queues are busy with x)
            scr = const.tile([128, 512], BF16, tag="scr")
            nc.vector.memset(scr[:], 0.0)

            # scatter masks from iota: Lmod[j, m] = 1 iff LCOL[j] % 128 == m
            lmod = const.tile([128, 128], BF16, tag="lmod")
            nc.vector.tensor_scalar(lmod[:], io_sb, v_sb[:, 8:9],
                                    None, op0=ALU.is_equal)
            rmod = const.tile([128, 128], BF16, tag="rmod")
            nc.vector.tensor_scalar(rmod[:], io_sb, v_sb[:, 9:10],
                                    None, op0=ALU.is_equal)

            # ---------- aux compute: dd[j] = sum_i |w[i,d_j] - old[i,d_j]| --
            dch = aux.tile([128, NIN], F32, tag="dch")
            nc.vector.tensor_tensor(dch[:], wd_sb, od_sb, op=ALU.subtract)
            dd = const.tile([128, 1], F32, tag="dd")
            nc.vector.tensor_reduce(
                dd[:], dch[:], axis=mybir.AxisListType.X, op=ALU.add,
                apply_absolute_value=True,
            )
            # active = (dd > THRESHOLD) & ((batch_ctr - indicator) > REF_PERIOD)
            t1 = const.tile([128, 1], F32, tag="t1")
            nc.vector.tensor_tensor(t1[:], v_sb[:, 3:4], v_sb[:, 2:3],
                                    op=ALU.subtract)
            c2 = const.tile([128, 1], F32, tag="c2")
            nc.vector.tensor_scalar(c2[:], t1[:], REF_PERIOD, None, op0=ALU.is_gt)
            c1 = const.tile([128, 1], F32, tag="c1")
            nc.vector.tensor_scalar(c1[:], dd[:], THRESHOLD, None, op0=ALU.is_gt)
            av = const.tile([128, 1], F32, tag="av")
            nc.vector.tensor_tensor(av[:], c1[:], c2[:], op=ALU.mult)
            da = const.tile([128, 1], F32, tag="da")
            nc.vector.tensor_tensor(da[:], dd[:], av[:], op=ALU.mult)
            lf1 = const.tile([128, 1], F32, tag="lf1")
            nc.vector.tensor_tensor(lf1[:], da[:], v_sb[:, 0:1], op=ALU.mult)
            rf1 = const.tile([128, 1], F32, tag="rf1")
            nc.vector.tensor_tensor(rf1[:], da[:], v_sb[:, 1:2], op=ALU.mult)

            lfc = const.tile([128, CC], BF16, tag="lfc")
            nc.vector.tensor_scalar(lfc[:], v_sb[:, 10:10 + CC], lf1[:],
                                    None, op0=ALU.mult)
            rfc = const.tile([128, CC], BF16, tag="rfc")
            nc.vector.tensor_scalar(rfc[:], v_sb[:, 14:14 + CC], rf1[:],
                                    None, op0=ALU.mult)
            multm = const.tile([128, CC], F32, tag="multm")
            mult_sb = [multm[:, cc:cc + 1] for cc in range(CC)]

            # ---------- main: y^T = (w^T x^T) scaled+biased+relu ----------
            def evict_act(ps, ob, c, half):
                o = c * W + half * 512
                nc.scalar.activation(
                    ob[:, o:o + 512], ps[:], AF.Relu,
                    bias=v_sb[:, 4 + c:5 + c], scale=mult_sb[c])

            def evict_dve(ps, ob, c, half):
                o = c * W + half * 512
                tmp = tpool.tile([128, 512], F32, tag="evt")
                nc.vector.tensor_scalar(
                    tmp[:], ps[:], mult_sb[c], v_sb[:, 4 + c:5 + c],
                    op0=ALU.mult, op1=ALU.add)
                nc.vector.tensor_scalar(
                    ob[:, o:o + 512], tmp[:], 0.0, None, op0=ALU.max)

            with tc.tile_pool(name="ps", bufs=1, space="PSUM") as pspool:
                # PE clock warm-up burst bridges until the first x chunk
                warm = pspool.tile([128, 512], F32, tag="m5", bufs=6)
                for _ in range(NWARM):
                    nc.tensor.matmul(warm[:], scr[:, :128], scr[:],
                                     start=True, stop=True)

                def aux_scatter():
                    # dop scatter matmuls; tiny separate psum tag so the
                    # mult critical path never touches the main rotation
                    psl = pspool.tile([128, CC], F32, tag="auxps", bufs=2)
                    nc.tensor.matmul(psl[:], lmod[:], lfc[:],
                                     start=True, stop=True)
                    psr = pspool.tile([128, CC], F32, tag="auxps", bufs=2)
                    nc.tensor.matmul(psr[:], rmod[:], rfc[:],
                                     start=True, stop=True)
                    lsp = const.tile([128, CC], F32, tag="lsp")
                    nc.vector.tensor_scalar(lsp[:], psl[:], 1.0, None,
                                            op0=ALU.add)
                    rsp = const.tile([128, CC], F32, tag="rsp")
                    nc.vector.tensor_scalar(rsp[:], psr[:], 1.0, None,
                                            op0=ALU.add)
                    nc.vector.tensor_tensor(multm[:], lsp[:], rsp[:],
                                            op=ALU.mult)

                def solo(wp, half, first=False, yengs=None):
                    # k-outer half-window: 4 open [128,512] psum groups;
                    # consumes one 128KB x chunk per 4 matmuls -- matches
                    # what the DMA queues can deliver during their ramp
                    xa = xa_tiles[wp]
                    if wp not in solo_ob:
                        solo_ob[wp] = opool.tile([128, CC * W], BF16,
                                                 tag="ob", name=f"sob{wp}")
                    ob = solo_ob[wp]
                    pcs = [pspool.tile([128, 512], F32, tag="m5", bufs=6,
                                       name=f"pc{wp}_{half}_{c}")
                           for c in range(CC)]
                    for ki in range(KC):
                        for c in range(CC):
                            nc.tensor.matmul(
                                pcs[c][:], wk_tile(ki, c),
                                xa[:, ki * W + half * 512:
                                   ki * W + half * 512 + 512],
                                start=(ki == 0), stop=(ki == KC - 1))
                        if first and ki == KC - 2:
                            aux_scatter()
                    for c in range(CC):
                        if c % 2 == 0:
                            evict_act(pcs[c], ob, c, half)
                        else:
                            evict_dve(pcs[c], ob, c, half)
                        o = c * W + half * 512
                        yengs[c].dma_start(yt[wp][:, o:o + 512],
                                           ob[:, o:o + 512])

                # tail window: c-outer over half-pairs -- psum groups stop
                # staggered, so only one small eviction trails the last mm
                def tailwin(wp):
                    xa = xa_tiles[wp]
                    ob = opool.tile([128, CC * W], BF16, tag="ob",
                                    name=f"tob{wp}")
                    for c in range(CC):
                        ps0 = pspool.tile([128, 512], F32, tag="m5", bufs=6,
                                          name=f"tp{wp}_{c}_0")
                        ps1 = pspool.tile([128, 512], F32, tag="m5", bufs=6,
                                          name=f"tp{wp}_{c}_1")
                        korder = range(KC) if c % 2 == 0 \
                            else range(KC - 1, -1, -1)
                        for ki, k in enumerate(korder):
                            for s, ps in ((0, ps0), (1, ps1)):
                                nc.tensor.matmul(
                                    ps[:], wk_tile(k, c),
                                    xa[:, k * W + s * 512:
                                       k * W + (s + 1) * 512],
                                    start=(ki == 0), stop=(ki == KC - 1))
                        evict_act(ps0, ob, c, 0)
                        o = c * W
                        eng = nc.scalar if c % 2 == 0 else nc.sync
                        eng.dma_start(yt[wp][:, o:o + 512], ob[:, o:o + 512])
                        evict_dve(ps1, ob, c, 1)
                        eng2 = nc.sync if c % 2 == 0 else nc.scalar
                        eng2.dma_start(yt[wp][:, o + 512:o + W],
                                       ob[:, o + 512:o + W])

                solo_ob = {}
                # y drains: early/mid solos alternate sync/gpsimd; late
                # solos move off gpsimd (slow SWDGE end-drain)
                YEG = {0: nc.sync, 1: nc.gpsimd, 2: nc.sync, 3: nc.gpsimd}
                YLATE = {0: nc.scalar, 1: nc.sync, 2: nc.scalar, 3: nc.sync}
                # x prefetch pattern: window wp's x is issued ~3 windows
                # ahead, halves split across two queues
                XENG = {1: (nc.sync, nc.gpsimd), 2: (nc.sync, nc.gpsimd),
                        3: (nc.sync, nc.gpsimd), 4: (nc.scalar, nc.scalar),
                        5: (nc.scalar, nc.scalar), 6: (nc.sync, nc.gpsimd),
                        7: (nc.sync, nc.gpsimd)}

                solo(0, 0, first=True, yengs=YEG)
                load_xa(2, *XENG[2])
                solo(0, 1, yengs=YEG)
                load_xa(3, *XENG[3])
                solo(1, 0, yengs=YEG)
                load_xa(4, *XENG[4])
                solo(1, 1, yengs=YEG)
                load_xa(5, *XENG[5])
                solo(2, 0, yengs=YEG)
                solo(2, 1, yengs=YEG)
                load_xa(6, *XENG[6])
                solo(3, 0, yengs=YEG)
                solo(3, 1, yengs=YEG)
                load_xa(7, *XENG[7])
                solo(4, 0, yengs=YEG)
                solo(4, 1, yengs=YEG)
                solo(5, 0, yengs=YLATE)
                solo(5, 1, yengs=YLATE)
                solo(6, 0, yengs=YLATE)
                solo(6, 1, yengs=YLATE)
                tailwin(7)

    nc.compile()
    _CACHED_NC = nc
    return nc


LAST_RESULTS = None


def kernel(x, w, b, dop_weights_old, indicator, batch_ctr):
    global LAST_RESULTS
    x = np.asarray(x, dtype=np.float32)
    w = np.ascontiguousarray(np.asarray(w, dtype=np.float32))
    b_arr = np.asarray(b, dtype=np.float32)
    old = np.asarray(dop_weights_old, dtype=np.float32)
    ind = np.asarray(indicator, dtype=np.float32)
    bc_val = float(np.asarray(batch_ctr).item())

    nc = build_nc()

    # replicated (per-core identical) inputs; all reshapes/gathers are pure
    # data marshaling -- every arithmetic op happens on device
    w4 = w.reshape(KC, 128, CC, 128)
    wkb = np.ascontiguousarray(np.concatenate(
        [w4[k, :, c, :] for (k, c) in ORDER], axis=1)).astype(BF16_NP)
    vcols = [LOK10, ROK10, ind.astype(np.float32),
             np.full(128, bc_val, np.float32)]
    vcols += [b_arr[c * 128:(c + 1) * 128] for c in range(CC)]
    vcols += [(LCOL % 128).astype(np.float32), (RCOL % 128).astype(np.float32)]
    vcols += [(LCOL // 128 == cc).astype(np.float32) for cc in range(CC)]
    vcols += [(RCOL // 128 == cc).astype(np.float32) for cc in range(CC)]
    vecs = np.stack(vcols, axis=1).astype(np.float32)
    iot = np.broadcast_to(np.arange(128, dtype=np.float32), (128, 128))
    auxs = np.ascontiguousarray(np.concatenate(
        [vecs, iot], axis=1, dtype=np.float32))
    auxb = np.ascontiguousarray(np.concatenate(
        [w.T[DOP_IDX], old.T[DOP_IDX]], axis=1, dtype=np.float32)
    ).astype(BF16_NP)

    common = dict(wkb=wkb, auxs=auxs, auxb=auxb)

    xbf = x.astype(BF16_NP)
    in_maps = []
    for i in range(N_CORES):
        xs = xbf[i * SHARD:(i + 1) * SHARD]          # [8192, 512]
        xtc = np.ascontiguousarray(
            xs.reshape(NWP, W, KC, 128).transpose(0, 3, 2, 1)
        ).reshape(NWP, 128, KC * W)
        in_maps.append(dict(common, xt=xtc))

    res = run_bass_kernel_spmd(nc, in_maps, core_ids=list(range(N_CORES)))
    LAST_RESULTS = res

    out = np.empty((B, UNITS), np.float32)
    for i in range(N_CORES):
        ytc = res.results[i]["yt"].astype(np.float32).reshape(NWP, 128, CC, W)
        out[i * SHARD:(i + 1) * SHARD] = (
            ytc.transpose(0, 3, 2, 1).reshape(SHARD, UNITS))
    return out
